# revision 13
# baseline (speedup 1.0000x reference)
"""DeepSet GNN message-passing kernel for 8 TRN2 NeuronCores.

Strategy (v3):
  - segment_ids are sorted; segments are dealt round-robin BY SIZE to the 8
    cores so every core sees a near-identical histogram of segment sizes ->
    the shared SPMD column schedule has ~2% padding instead of ~10%.
  - All-transposed dataflow: device computes h^T = relu(W1^T X^T + b1) with
    a block-diagonal [[W1,0],[0,W1]] stationary on the PE; two independent
    column streams live on partition halves 0-63 / 64-127, so each 512-wide
    matmul handles 1024 neighbors.
  - PSUM->SBUF relu evacuation is split between ACT (3/4) and DVE (1/4).
  - The per-segment sum is a compile-time schedule of windowed reductions:
    segments are grouped by exact size (slot lengths padded to even), the
    hottest classes get a GpSimd pairwise pre-add (halving DVE work; L=2
    classes finish entirely on GpSimd), the rest reduce directly on DVE.
  - rho MLP: phi_w2 folded past the segment sum (waa = phi_w2 @ rho_w1);
    count * (phi_b2 @ rho_w1) and rho_b2 enter via rank-1 matmuls; rho_b1
    via ACT per-partition bias. Output written transposed; host un-permutes.
"""

import sys

sys.path.insert(0, "/opt/trn_rl_repo")

import numpy as np

N_AGENTS = 50000
N_NEIGH = 1600000
D = 64
N_CORES = 8
SPC = N_AGENTS // N_CORES  # 6250 segments per core
TILE_COLS = 2048  # columns per DMA/hs tile (per half-stream)
SUB = 512  # columns per matmul / PSUM bank
RHO_W = 512  # segments per rho window

EVAC_DVE = {3}  # which 512-col subchunks DVE evacuates (of 4 per tile)
GP_PAIR_NS = 1.0  # assumed GpSimd pair-add cost per original column (ns)
GP_BUDGET_NS = 75000.0  # GpSimd load target per core

LAST_RESULTS = None


def _make_schedule(counts, even_pad):
    """counts: [N_CORES, SPC_like] per-core segment sizes (class-balanced).
    Build the shared column schedule: per distinct size k (ascending),
    ceil(max_core_count(k)/2) slots of length L_k; slots never cross a
    TILE_COLS boundary. Classes are assigned a reduction mode:
      A: DVE tensor_reduce window L
      B: GpSimd pair-add then DVE tensor_reduce window L/2
      C: (L==2) GpSimd pair-add straight into ssb
    Returns per-tile run lists plus slot metadata."""
    KMAX = int(counts.max())
    n_prog = np.zeros(KMAX + 1, np.int64)
    for c in range(N_CORES):
        bc = np.bincount(counts[c], minlength=KMAX + 1)
        n_prog = np.maximum(n_prog, bc)

    def L_of(k):
        if even_pad:
            return max(2, k + (k & 1))
        return max(1, k)

    classes = [k for k in range(KMAX + 1) if n_prog[k] > 0]
    W = {k: -(-int(n_prog[k]) // 2) for k in classes}
    # mode assignment: hottest (largest L first) classes to GpSimd
    mode = {}
    gp_load = 0.0
    for k in sorted(classes, key=lambda k: -L_of(k)):
        L = L_of(k)
        C_k = W[k] * L
        if L % 2 == 0 and gp_load + C_k * GP_PAIR_NS <= GP_BUDGET_NS:
            mode[k] = "C" if L == 2 else "B"
            gp_load += C_k * GP_PAIR_NS
        else:
            mode[k] = "A"

    pair_runs = {}  # tile -> [(off, ncols, dst0, to_ssb)]
    red_runs = {}  # tile -> [(src_h2, off, n, Lw, slot0)]
    slot_col = []
    cls_slot0 = {}
    col = 0
    for k in classes:
        L = L_of(k)
        cls_slot0[k] = len(slot_col)
        left = W[k]
        while left:
            space = TILE_COLS - (col % TILE_COLS)
            nfit = min(left, space // L)
            if nfit == 0:
                col += space
                continue
            t = col // TILE_COLS
            off = col - t * TILE_COLS
            slot0 = len(slot_col)
            if mode[k] == "A":
                red_runs.setdefault(t, []).append((False, off, nfit, L, slot0))
            elif mode[k] == "C":
                pair_runs.setdefault(t, []).append((off, nfit * L, slot0, True))
            else:  # B
                pair_runs.setdefault(t, []).append((off, nfit * L, off // 2, False))
                red_runs.setdefault(t, []).append(
                    (True, off // 2, nfit, L // 2, slot0)
                )
            slot_col.extend((col + L * np.arange(nfit)).tolist())
            col += nfit * L
            left -= nfit
    NSLOT = len(slot_col)
    NSLOT_pad = -(-NSLOT // RHO_W) * RHO_W
    NTILES = -(-col // TILE_COLS)
    S = NTILES * TILE_COLS
    Lmap = {k: L_of(k) for k in classes}
    return (pair_runs, red_runs, np.asarray(slot_col, np.int64), cls_slot0,
            Lmap, NSLOT, NSLOT_pad, S, NTILES)


def _build_program(pair_runs, red_runs, NTILES, NSLOT_pad, S):
    from concourse import bacc, mybir
    import concourse.tile as tile

    FP16 = mybir.dt.float16
    F32 = mybir.dt.float32
    Relu = mybir.ActivationFunctionType.Relu
    Copy = mybir.ActivationFunctionType.Copy
    AXX = mybir.AxisListType.X
    ADD = mybir.AluOpType.add
    MAX = mybir.AluOpType.max

    nc = bacc.Bacc("TRN2", target_bir_lowering=False, debug=False)
    xd = nc.dram_tensor("xd", [128, S], FP16, kind="ExternalInput").ap()
    cntd = nc.dram_tensor("cntd", [65, NSLOT_pad], FP16, kind="ExternalInput").ap()
    w2sd = nc.dram_tensor("w2sd", [128, 128], FP16, kind="ExternalInput").ap()
    b1d = nc.dram_tensor("b1d", [128, 1], F32, kind="ExternalInput").ap()
    waad = nc.dram_tensor("waad", [128, 64], FP16, kind="ExternalInput").ap()
    b2rd = nc.dram_tensor("b2rd", [65, 64], FP16, kind="ExternalInput").ap()
    rb1d = nc.dram_tensor("rb1d", [64, 1], F32, kind="ExternalInput").ap()
    rw2d = nc.dram_tensor("rw2d", [64, 2], FP16, kind="ExternalInput").ap()
    rb2d = nc.dram_tensor("rb2d", [1, 2], FP16, kind="ExternalInput").ap()
    onesd = nc.dram_tensor("onesd", [1, RHO_W], FP16, kind="ExternalInput").ap()
    outd = nc.dram_tensor("outd", [4, NSLOT_pad], F32, kind="ExternalOutput").ap()

    NWIN = NSLOT_pad // RHO_W
    with tile.TileContext(nc) as tc:
        with (
            tc.tile_pool(name="const", bufs=1) as cpool,
            tc.tile_pool(name="x", bufs=3) as xpool,
            tc.tile_pool(name="h", bufs=3) as hpool,
            tc.tile_pool(name="h2", bufs=3) as h2pool,
            tc.tile_pool(name="r", bufs=2) as rpool,
            tc.tile_pool(name="ph", bufs=4, space="PSUM") as ph,
            tc.tile_pool(name="pr", bufs=2, space="PSUM") as pr,
            tc.tile_pool(name="po", bufs=2, space="PSUM") as po,
        ):
            w2s_t = cpool.tile([128, 128], FP16)
            nc.sync.dma_start(w2s_t[:], w2sd[:, :])
            b1_t = cpool.tile([128, 1], F32)
            nc.sync.dma_start(b1_t[:], b1d[:, :])
            waa_t = cpool.tile([128, 64], FP16)
            nc.sync.dma_start(waa_t[:], waad[:, :])
            b2r_t = cpool.tile([65, 64], FP16)
            nc.sync.dma_start(b2r_t[:], b2rd[:, :])
            rb1_t = cpool.tile([64, 1], F32)
            nc.sync.dma_start(rb1_t[:], rb1d[:, :])
            rw2_t = cpool.tile([64, 2], FP16)
            nc.sync.dma_start(rw2_t[:], rw2d[:, :])
            rb2_t = cpool.tile([1, 2], FP16)
            nc.sync.dma_start(rb2_t[:], rb2d[:, :])
            ones_t = cpool.tile([1, RHO_W], FP16)
            nc.sync.dma_start(ones_t[:], onesd[:, :])
            cnt_t = cpool.tile([65, NSLOT_pad], FP16)
            nc.sync.dma_start(cnt_t[:], cntd[:, :])
            ssb = cpool.tile([128, NSLOT_pad], FP16)
            outsb = cpool.tile([66, NSLOT_pad], F32)
            nc.gpsimd.memset(ssb[:], 0.0)

            with nc.allow_low_precision(reason="fp16 segment sums within tol"):
                for t in range(NTILES):
                    xt = xpool.tile([128, TILE_COLS], FP16)
                    nc.sync.dma_start(
                        xt[:], xd[:, t * TILE_COLS : (t + 1) * TILE_COLS]
                    )
                    hst = hpool.tile([128, TILE_COLS], FP16)
                    h2t = h2pool.tile([128, TILE_COLS // 2], FP16)
                    for j in range(TILE_COLS // SUB):
                        hp = ph.tile([128, SUB], F32)
                        nc.tensor.matmul(
                            hp[:],
                            lhsT=w2s_t[:],
                            rhs=xt[:, j * SUB : (j + 1) * SUB],
                            start=True,
                            stop=True,
                        )
                        dst = hst[:, j * SUB : (j + 1) * SUB]
                        if j in EVAC_DVE:
                            nc.vector.tensor_scalar(
                                out=dst, in0=hp[:], scalar1=b1_t[:],
                                scalar2=0.0, op0=ADD, op1=MAX,
                            )
                        else:
                            nc.scalar.activation(dst, hp[:], Relu, bias=b1_t[:])
                    for (off, ncols, dst0, to_ssb) in pair_runs.get(t, []):
                        i0 = hst[:, off : off + ncols : 2]
                        i1 = hst[:, off + 1 : off + ncols : 2]
                        out_ap = (
                            ssb[:, dst0 : dst0 + ncols // 2]
                            if to_ssb
                            else h2t[:, dst0 : dst0 + ncols // 2]
                        )
                        nc.gpsimd.tensor_tensor(
                            out=out_ap, in0=i0, in1=i1, op=ADD
                        )
                    for (src_h2, off, n, Lw, slot0) in red_runs.get(t, []):
                        src = h2t if src_h2 else hst
                        nc.vector.tensor_reduce(
                            ssb[:, slot0 : slot0 + n],
                            src[:, off : off + n * Lw].rearrange(
                                "p (n l) -> p n l", l=Lw
                            ),
                            axis=AXX,
                            op=ADD,
                        )
                for w in range(NWIN):
                    c0 = w * RHO_W
                    for h in range(2):
                        rp = pr.tile([64, RHO_W], F32)
                        nc.tensor.matmul(
                            rp[:], lhsT=waa_t[64 * h : 64 * h + 64, :],
                            rhs=ssb[64 * h : 64 * h + 64, c0 : c0 + RHO_W],
                            start=True, stop=False,
                        )
                        nc.tensor.matmul(
                            rp[:], lhsT=b2r_t[64 * h : 64 * h + 1, :],
                            rhs=cnt_t[64 * h : 64 * h + 1, c0 : c0 + RHO_W],
                            start=False, stop=True,
                        )
                        rsb = rpool.tile([64, RHO_W], FP16)
                        nc.scalar.activation(rsb[:], rp[:], Relu, bias=rb1_t[:])
                        op_ = po.tile([2, RHO_W], F32)
                        nc.tensor.matmul(
                            op_[:], lhsT=rw2_t[:], rhs=rsb[:],
                            start=True, stop=False,
                        )
                        nc.tensor.matmul(
                            op_[:], lhsT=rb2_t[:], rhs=ones_t[:],
                            start=False, stop=True,
                        )
                        nc.scalar.activation(
                            outsb[64 * h : 64 * h + 2, c0 : c0 + RHO_W], op_[:],
                            Copy,
                        )
                    nc.sync.dma_start(
                        outd[0:2, c0 : c0 + RHO_W], outsb[0:2, c0 : c0 + RHO_W]
                    )
                    nc.sync.dma_start(
                        outd[2:4, c0 : c0 + RHO_W], outsb[64:66, c0 : c0 + RHO_W]
                    )
    nc.compile()
    return nc


def _host_prep(neighbors, phi_w1, phi_b1, phi_w2, phi_b2,
               rho_w1, rho_b1, rho_w2, rho_b2, segment_ids):
    ids = np.asarray(segment_ids)
    X16 = np.asarray(neighbors).astype(np.float16)

    counts_all = np.bincount(ids, minlength=N_AGENTS)
    row_start = np.concatenate([[0], np.cumsum(counts_all)])
    # deal segments round-robin by size so core class-histograms match
    order_all = np.argsort(counts_all, kind="stable")
    core_segs = [order_all[c::N_CORES] for c in range(N_CORES)]
    maxn = max(len(s) for s in core_segs)
    counts = np.zeros((N_CORES, maxn), np.int64)
    for c in range(N_CORES):
        counts[c, : len(core_segs[c])] = counts_all[core_segs[c]]

    even_pad = bool(np.all(np.asarray(phi_b1) == 0))
    sched = _make_schedule(counts, even_pad)
    (pair_runs, red_runs, slot_col, cls_slot0, Lmap,
     NSLOT, NSLOT_pad, S, NTILES) = sched

    w2s = np.zeros((128, 128), np.float16)
    w2s[0:64, 0:64] = phi_w1
    w2s[64:128, 64:128] = phi_w1
    consts = dict(
        w2sd=w2s,
        b1d=np.tile(np.asarray(phi_b1, np.float32)[:, None], (2, 1)),
        waad=np.tile((np.asarray(phi_w2) @ np.asarray(rho_w1)).astype(np.float16),
                     (2, 1)),
        b2rd=np.zeros((65, 64), np.float16),
        rb1d=np.asarray(rho_b1, np.float32)[:, None],
        rw2d=np.asarray(rho_w2).astype(np.float16),
        rb2d=np.asarray(rho_b2)[None, :].astype(np.float16),
        onesd=np.ones((1, RHO_W), np.float16),
    )
    b2row = (np.asarray(phi_b2) @ np.asarray(rho_w1)).astype(np.float16)
    consts["b2rd"][0] = b2row
    consts["b2rd"][64] = b2row

    in_maps = []
    slotmaps = []
    for c in range(N_CORES):
        segs_c = core_segs[c]  # ascending by count (argsort order)
        k_c = counts_all[segs_c]
        uniq, first = np.unique(k_c, return_index=True)
        xdv = np.zeros((128, S), np.float16)
        cnt = np.zeros((65, NSLOT_pad), np.float16)
        smap = np.full((2, NSLOT_pad), -1, np.int64)
        for h in (0, 1):
            seg_list, slot_list = [], []
            for i, kv in enumerate(uniq):
                s0 = first[i]
                s1 = first[i + 1] if i + 1 < len(uniq) else len(segs_c)
                segs = segs_c[s0:s1]
                nE = (len(segs) + 1) // 2
                mine = segs[:nE] if h == 0 else segs[nE:]
                if len(mine) == 0:
                    continue
                base = cls_slot0[int(kv)]
                seg_list.append(mine)
                slot_list.append(base + np.arange(len(mine)))
            segs_f = np.concatenate(seg_list)
            slots_f = np.concatenate(slot_list)
            kk = counts_all[segs_f]
            nz = kk > 0
            segs_nz, slots_nz, kknz = segs_f[nz], slots_f[nz], kk[nz]
            src0 = row_start[segs_nz]
            col0 = slot_col[slots_nz]
            tot = int(kknz.sum())
            ofs = np.arange(tot) - np.repeat(np.cumsum(kknz) - kknz, kknz)
            src = np.repeat(src0, kknz) + ofs
            dst = np.repeat(col0, kknz) + ofs
            xdv[64 * h : 64 * h + 64, dst] = X16[src].T
            cnt[64 * h, slots_f] = kk.astype(np.float16)
            smap[h, slots_f] = segs_f
        in_maps.append(dict(xd=xdv, cntd=cnt, **consts))
        slotmaps.append(smap)
    return sched, in_maps, slotmaps


def kernel(**inputs):
    global LAST_RESULTS
    np_inputs = {kk: np.asarray(v) for kk, v in inputs.items()}
    sched, in_maps, slotmaps = _host_prep(**np_inputs)
    (pair_runs, red_runs, slot_col, cls_slot0, Lmap,
     NSLOT, NSLOT_pad, S, NTILES) = sched
    nc = _build_program(pair_runs, red_runs, NTILES, NSLOT_pad, S)

    from concourse.bass_utils import run_bass_kernel_spmd

    res = run_bass_kernel_spmd(nc, in_maps, list(range(N_CORES)))
    LAST_RESULTS = res

    y = np.zeros((N_AGENTS, 2), np.float32)
    for c in range(N_CORES):
        o = res.results[c]["outd"]
        sm = slotmaps[c]
        for h in (0, 1):
            m = sm[h] >= 0
            y[sm[h][m]] = o[2 * h : 2 * h + 2, m].T
    return y


# revision 14
# speedup vs baseline: 1.1761x; 1.1761x over previous
"""DeepSet GNN message-passing kernel for 8 TRN2 NeuronCores.

Strategy (v3):
  - segment_ids are sorted; segments are dealt round-robin BY SIZE to the 8
    cores so every core sees a near-identical histogram of segment sizes ->
    the shared SPMD column schedule has ~2% padding instead of ~10%.
  - All-transposed dataflow: device computes h^T = relu(W1^T X^T + b1) with
    a block-diagonal [[W1,0],[0,W1]] stationary on the PE; two independent
    column streams live on partition halves 0-63 / 64-127, so each 512-wide
    matmul handles 1024 neighbors.
  - PSUM->SBUF relu evacuation is split between ACT (3/4) and DVE (1/4).
  - The per-segment sum is a compile-time schedule of windowed reductions:
    segments are grouped by exact size (slot lengths padded to even), the
    hottest classes get a GpSimd pairwise pre-add (halving DVE work; L=2
    classes finish entirely on GpSimd), the rest reduce directly on DVE.
  - rho MLP: phi_w2 folded past the segment sum (waa = phi_w2 @ rho_w1);
    count * (phi_b2 @ rho_w1) and rho_b2 enter via rank-1 matmuls; rho_b1
    via ACT per-partition bias. Output written transposed; host un-permutes.
"""

import sys

sys.path.insert(0, "/opt/trn_rl_repo")

import numpy as np

N_AGENTS = 50000
N_NEIGH = 1600000
D = 64
N_CORES = 8
SPC = N_AGENTS // N_CORES  # 6250 segments per core
TILE_COLS = 2048  # columns per DMA/hs tile (per half-stream)
SUB = 512  # columns per matmul / PSUM bank
RHO_W = 512  # segments per rho window

EVAC_DVE = {3}  # which 512-col subchunks DVE evacuates (of 4 per tile)
GP_PAIR_NS = 1.21  # measured GpSimd pair-add cost per original column (ns)
GP_BUDGET_NS = 85000.0  # GpSimd load target per core

LAST_RESULTS = None


def _make_schedule(counts, even_pad):
    """counts: [N_CORES, SPC_like] per-core segment sizes (class-balanced).
    Build the shared column schedule: per distinct size k (ascending),
    ceil(max_core_count(k)/2) slots of length L_k; slots never cross a
    TILE_COLS boundary. Classes are assigned a reduction mode:
      A: DVE tensor_reduce window L
      B: GpSimd pair-add then DVE tensor_reduce window L/2
      C: (L==2) GpSimd pair-add straight into ssb
    Returns per-tile run lists plus slot metadata."""
    KMAX = int(counts.max())
    n_prog = np.zeros(KMAX + 1, np.int64)
    for c in range(N_CORES):
        bc = np.bincount(counts[c], minlength=KMAX + 1)
        n_prog = np.maximum(n_prog, bc)

    def L_of(k):
        if even_pad:
            return max(2, k + (k & 1))
        return max(1, k)

    classes = [k for k in range(KMAX + 1) if n_prog[k] > 0]
    W = {k: -(-int(n_prog[k]) // 2) for k in classes}
    # mode assignment: hottest (largest L first) classes to GpSimd
    mode = {}
    gp_load = 0.0
    for k in sorted(classes, key=lambda k: -L_of(k)):
        L = L_of(k)
        C_k = W[k] * L
        if L % 2 == 0 and gp_load + C_k * GP_PAIR_NS <= GP_BUDGET_NS:
            mode[k] = "C" if L == 2 else "B"
            gp_load += C_k * GP_PAIR_NS
        else:
            mode[k] = "A"

    pair_runs = {}  # tile -> [(off, ncols, dst0, to_ssb)]
    red_runs = {}  # tile -> [(src_h2, off, n, Lw, slot0)]
    slot_col = []
    cls_slot0 = {}
    col = 0
    for k in classes:
        L = L_of(k)
        cls_slot0[k] = len(slot_col)
        left = W[k]
        while left:
            space = TILE_COLS - (col % TILE_COLS)
            nfit = min(left, space // L)
            if nfit == 0:
                col += space
                continue
            t = col // TILE_COLS
            off = col - t * TILE_COLS
            slot0 = len(slot_col)
            if mode[k] == "A":
                red_runs.setdefault(t, []).append((False, off, nfit, L, slot0))
            elif mode[k] == "C":
                pair_runs.setdefault(t, []).append((off, nfit * L, slot0, True))
            else:  # B
                pair_runs.setdefault(t, []).append((off, nfit * L, off // 2, False))
                red_runs.setdefault(t, []).append(
                    (True, off // 2, nfit, L // 2, slot0)
                )
            slot_col.extend((col + L * np.arange(nfit)).tolist())
            col += nfit * L
            left -= nfit
    NSLOT = len(slot_col)
    NSLOT_pad = -(-NSLOT // RHO_W) * RHO_W
    NTILES = -(-col // TILE_COLS)
    S = NTILES * TILE_COLS
    Lmap = {k: L_of(k) for k in classes}
    return (pair_runs, red_runs, np.asarray(slot_col, np.int64), cls_slot0,
            Lmap, NSLOT, NSLOT_pad, S, NTILES)


def _build_program(pair_runs, red_runs, NTILES, NSLOT_pad, S):
    from concourse import bacc, mybir
    import concourse.tile as tile

    FP16 = mybir.dt.float16
    F32 = mybir.dt.float32
    Relu = mybir.ActivationFunctionType.Relu
    Copy = mybir.ActivationFunctionType.Copy
    Identity = mybir.ActivationFunctionType.Identity
    AXX = mybir.AxisListType.X
    ADD = mybir.AluOpType.add
    MAX = mybir.AluOpType.max

    nc = bacc.Bacc("TRN2", target_bir_lowering=False, debug=False)
    xd = nc.dram_tensor("xd", [128, S], FP16, kind="ExternalInput").ap()
    cntd = nc.dram_tensor("cntd", [65, NSLOT_pad], FP16, kind="ExternalInput").ap()
    w2sd = nc.dram_tensor("w2sd", [128, 128], FP16, kind="ExternalInput").ap()
    b1d = nc.dram_tensor("b1d", [128, 1], F32, kind="ExternalInput").ap()
    waad = nc.dram_tensor("waad", [128, 64], FP16, kind="ExternalInput").ap()
    b2rd = nc.dram_tensor("b2rd", [65, 64], FP16, kind="ExternalInput").ap()
    rb1d = nc.dram_tensor("rb1d", [64, 1], F32, kind="ExternalInput").ap()
    rw2d = nc.dram_tensor("rw2d", [64, 2], FP16, kind="ExternalInput").ap()
    rb2d = nc.dram_tensor("rb2d", [2, 1], F32, kind="ExternalInput").ap()
    outd = nc.dram_tensor("outd", [4, NSLOT_pad], F32, kind="ExternalOutput").ap()

    NWIN = NSLOT_pad // RHO_W
    with tile.TileContext(nc) as tc:
        with (
            tc.tile_pool(name="const", bufs=1) as cpool,
            tc.tile_pool(name="x", bufs=3) as xpool,
            tc.tile_pool(name="h", bufs=3) as hpool,
            tc.tile_pool(name="h2", bufs=3) as h2pool,
            tc.tile_pool(name="r", bufs=2) as rpool,
            tc.tile_pool(name="ph", bufs=3, space="PSUM") as ph,
            tc.tile_pool(name="pr", bufs=1, space="PSUM") as pr,
            tc.tile_pool(name="po", bufs=1, space="PSUM") as po,
        ):
            w2s_t = cpool.tile([128, 128], FP16)
            nc.sync.dma_start(w2s_t[:], w2sd[:, :])
            b1_t = cpool.tile([128, 1], F32)
            nc.sync.dma_start(b1_t[:], b1d[:, :])
            waa_t = cpool.tile([128, 64], FP16)
            nc.sync.dma_start(waa_t[:], waad[:, :])
            b2r_t = cpool.tile([65, 64], FP16)
            nc.sync.dma_start(b2r_t[:], b2rd[:, :])
            rb1_t = cpool.tile([64, 1], F32)
            nc.sync.dma_start(rb1_t[:], rb1d[:, :])
            rw2_t = cpool.tile([64, 2], FP16)
            nc.sync.dma_start(rw2_t[:], rw2d[:, :])
            rb2_t = cpool.tile([2, 1], F32)
            nc.sync.dma_start(rb2_t[:], rb2d[:, :])
            cnt_t = cpool.tile([65, NSLOT_pad], FP16)
            nc.sync.dma_start(cnt_t[:], cntd[:, :])
            ssb = cpool.tile([128, NSLOT_pad], FP16)
            outsb = cpool.tile([66, NSLOT_pad], F32)
            nc.gpsimd.memset(ssb[:], 0.0)

            with nc.allow_low_precision(reason="fp16 segment sums within tol"):
                for t in range(NTILES):
                    xt = xpool.tile([128, TILE_COLS], FP16)
                    nc.sync.dma_start(
                        xt[:], xd[:, t * TILE_COLS : (t + 1) * TILE_COLS]
                    )
                    hst = hpool.tile([128, TILE_COLS], FP16)
                    h2t = h2pool.tile([128, TILE_COLS // 2], FP16)
                    for g in range(TILE_COLS // (2 * SUB)):
                        hp = ph.tile([128, 2 * SUB], F32)
                        for j in range(2):
                            nc.tensor.matmul(
                                hp[:, j * SUB : (j + 1) * SUB],
                                lhsT=w2s_t[:],
                                rhs=xt[:, (2 * g + j) * SUB : (2 * g + j + 1) * SUB],
                                start=True,
                                stop=True,
                            )
                        c0g = 2 * g * SUB
                        if g == 0:
                            nc.scalar.activation(
                                hst[:, c0g : c0g + 2 * SUB], hp[:], Relu,
                                bias=b1_t[:],
                            )
                        else:
                            nc.scalar.activation(
                                hst[:, c0g : c0g + SUB], hp[:, 0:SUB], Relu,
                                bias=b1_t[:],
                            )
                            nc.vector.tensor_scalar(
                                out=hst[:, c0g + SUB : c0g + 2 * SUB],
                                in0=hp[:, SUB : 2 * SUB], scalar1=b1_t[:],
                                scalar2=0.0, op0=ADD, op1=MAX,
                            )
                    for (off, ncols, dst0, to_ssb) in pair_runs.get(t, []):
                        i0 = hst[:, off : off + ncols : 2]
                        i1 = hst[:, off + 1 : off + ncols : 2]
                        out_ap = (
                            ssb[:, dst0 : dst0 + ncols // 2]
                            if to_ssb
                            else h2t[:, dst0 : dst0 + ncols // 2]
                        )
                        nc.gpsimd.tensor_tensor(
                            out=out_ap, in0=i0, in1=i1, op=ADD
                        )
                    for (src_h2, off, n, Lw, slot0) in red_runs.get(t, []):
                        src = h2t if src_h2 else hst
                        nc.vector.tensor_reduce(
                            ssb[:, slot0 : slot0 + n],
                            src[:, off : off + n * Lw].rearrange(
                                "p (n l) -> p n l", l=Lw
                            ),
                            axis=AXX,
                            op=ADD,
                        )
                for w in range(NWIN):
                    c0 = w * RHO_W
                    for h in range(2):
                        rp = pr.tile([64, RHO_W], F32)
                        nc.tensor.matmul(
                            rp[:], lhsT=waa_t[64 * h : 64 * h + 64, :],
                            rhs=ssb[64 * h : 64 * h + 64, c0 : c0 + RHO_W],
                            start=True, stop=False,
                        )
                        nc.tensor.matmul(
                            rp[:], lhsT=b2r_t[64 * h : 64 * h + 1, :],
                            rhs=cnt_t[64 * h : 64 * h + 1, c0 : c0 + RHO_W],
                            start=False, stop=True,
                        )
                        rsb = rpool.tile([64, RHO_W], FP16)
                        nc.scalar.activation(rsb[:], rp[:], Relu, bias=rb1_t[:])
                        op_ = po.tile([2, RHO_W], F32)
                        nc.tensor.matmul(
                            op_[:], lhsT=rw2_t[:], rhs=rsb[:],
                            start=True, stop=True,
                        )
                        nc.scalar.activation(
                            outsb[64 * h : 64 * h + 2, c0 : c0 + RHO_W], op_[:],
                            Identity, bias=rb2_t[:],
                        )
                    nc.sync.dma_start(
                        outd[0:2, c0 : c0 + RHO_W], outsb[0:2, c0 : c0 + RHO_W]
                    )
                    nc.sync.dma_start(
                        outd[2:4, c0 : c0 + RHO_W], outsb[64:66, c0 : c0 + RHO_W]
                    )
    nc.compile()
    return nc


def _host_prep(neighbors, phi_w1, phi_b1, phi_w2, phi_b2,
               rho_w1, rho_b1, rho_w2, rho_b2, segment_ids):
    ids = np.asarray(segment_ids)
    X16 = np.asarray(neighbors).astype(np.float16)

    counts_all = np.bincount(ids, minlength=N_AGENTS)
    row_start = np.concatenate([[0], np.cumsum(counts_all)])
    # deal segments round-robin by size so core class-histograms match
    order_all = np.argsort(counts_all, kind="stable")
    core_segs = [order_all[c::N_CORES] for c in range(N_CORES)]
    maxn = max(len(s) for s in core_segs)
    counts = np.zeros((N_CORES, maxn), np.int64)
    for c in range(N_CORES):
        counts[c, : len(core_segs[c])] = counts_all[core_segs[c]]

    even_pad = bool(np.all(np.asarray(phi_b1) == 0))
    sched = _make_schedule(counts, even_pad)
    (pair_runs, red_runs, slot_col, cls_slot0, Lmap,
     NSLOT, NSLOT_pad, S, NTILES) = sched

    w2s = np.zeros((128, 128), np.float16)
    w2s[0:64, 0:64] = phi_w1
    w2s[64:128, 64:128] = phi_w1
    consts = dict(
        w2sd=w2s,
        b1d=np.tile(np.asarray(phi_b1, np.float32)[:, None], (2, 1)),
        waad=np.tile((np.asarray(phi_w2) @ np.asarray(rho_w1)).astype(np.float16),
                     (2, 1)),
        b2rd=np.zeros((65, 64), np.float16),
        rb1d=np.asarray(rho_b1, np.float32)[:, None],
        rw2d=np.asarray(rho_w2).astype(np.float16),
        rb2d=np.asarray(rho_b2, np.float32)[:, None],
    )
    b2row = (np.asarray(phi_b2) @ np.asarray(rho_w1)).astype(np.float16)
    consts["b2rd"][0] = b2row
    consts["b2rd"][64] = b2row

    in_maps = []
    slotmaps = []
    for c in range(N_CORES):
        segs_c = core_segs[c]  # ascending by count (argsort order)
        k_c = counts_all[segs_c]
        uniq, first = np.unique(k_c, return_index=True)
        xdv = np.zeros((128, S), np.float16)
        cnt = np.zeros((65, NSLOT_pad), np.float16)
        smap = np.full((2, NSLOT_pad), -1, np.int64)
        for h in (0, 1):
            seg_list, slot_list = [], []
            for i, kv in enumerate(uniq):
                s0 = first[i]
                s1 = first[i + 1] if i + 1 < len(uniq) else len(segs_c)
                segs = segs_c[s0:s1]
                nE = (len(segs) + 1) // 2
                mine = segs[:nE] if h == 0 else segs[nE:]
                if len(mine) == 0:
                    continue
                base = cls_slot0[int(kv)]
                seg_list.append(mine)
                slot_list.append(base + np.arange(len(mine)))
            segs_f = np.concatenate(seg_list)
            slots_f = np.concatenate(slot_list)
            kk = counts_all[segs_f]
            nz = kk > 0
            segs_nz, slots_nz, kknz = segs_f[nz], slots_f[nz], kk[nz]
            src0 = row_start[segs_nz]
            col0 = slot_col[slots_nz]
            tot = int(kknz.sum())
            ofs = np.arange(tot) - np.repeat(np.cumsum(kknz) - kknz, kknz)
            src = np.repeat(src0, kknz) + ofs
            dst = np.repeat(col0, kknz) + ofs
            xdv[64 * h : 64 * h + 64, dst] = X16[src].T
            cnt[64 * h, slots_f] = kk.astype(np.float16)
            smap[h, slots_f] = segs_f
        in_maps.append(dict(xd=xdv, cntd=cnt, **consts))
        slotmaps.append(smap)
    return sched, in_maps, slotmaps


def kernel(**inputs):
    global LAST_RESULTS
    np_inputs = {kk: np.asarray(v) for kk, v in inputs.items()}
    sched, in_maps, slotmaps = _host_prep(**np_inputs)
    (pair_runs, red_runs, slot_col, cls_slot0, Lmap,
     NSLOT, NSLOT_pad, S, NTILES) = sched
    nc = _build_program(pair_runs, red_runs, NTILES, NSLOT_pad, S)

    from concourse.bass_utils import run_bass_kernel_spmd

    res = run_bass_kernel_spmd(nc, in_maps, list(range(N_CORES)))
    LAST_RESULTS = res

    y = np.zeros((N_AGENTS, 2), np.float32)
    for c in range(N_CORES):
        o = res.results[c]["outd"]
        sm = slotmaps[c]
        for h in (0, 1):
            m = sm[h] >= 0
            y[sm[h][m]] = o[2 * h : 2 * h + 2, m].T
    return y


# revision 17
# speedup vs baseline: 1.2317x; 1.0472x over previous
"""DeepSet GNN message-passing kernel for 8 TRN2 NeuronCores.

Strategy (v3):
  - segment_ids are sorted; segments are dealt round-robin BY SIZE to the 8
    cores so every core sees a near-identical histogram of segment sizes ->
    the shared SPMD column schedule has ~2% padding instead of ~10%.
  - All-transposed dataflow: device computes h^T = relu(W1^T X^T + b1) with
    a block-diagonal [[W1,0],[0,W1]] stationary on the PE; two independent
    column streams live on partition halves 0-63 / 64-127, so each 512-wide
    matmul handles 1024 neighbors.
  - PSUM->SBUF relu evacuation is split between ACT (3/4) and DVE (1/4).
  - The per-segment sum is a compile-time schedule of windowed reductions:
    segments are grouped by exact size (slot lengths padded to even), the
    hottest classes get a GpSimd pairwise pre-add (halving DVE work; L=2
    classes finish entirely on GpSimd), the rest reduce directly on DVE.
  - rho MLP: phi_w2 folded past the segment sum (waa = phi_w2 @ rho_w1);
    count * (phi_b2 @ rho_w1) and rho_b2 enter via rank-1 matmuls; rho_b1
    via ACT per-partition bias. Output written transposed; host un-permutes.
"""

import sys

sys.path.insert(0, "/opt/trn_rl_repo")

import numpy as np

N_AGENTS = 50000
N_NEIGH = 1600000
D = 64
N_CORES = 8
SPC = N_AGENTS // N_CORES  # 6250 segments per core
TILE_COLS = 2048  # columns per DMA/hs tile (per half-stream)
SUB = 512  # columns per matmul / PSUM bank
RHO_W = 512  # segments per rho window

EVAC_DVE = {3}  # which 512-col subchunks DVE evacuates (of 4 per tile)
GP_PAIR_NS = 1.21  # measured GpSimd pair-add cost per original column (ns)
GP_BUDGET_NS = 100000.0  # GpSimd load target per core

LAST_RESULTS = None


def _make_schedule(counts, even_pad):
    """counts: [N_CORES, SPC_like] per-core segment sizes (class-balanced).
    Build the shared column schedule: per distinct size k (ascending),
    ceil(max_core_count(k)/2) slots of length L_k; slots never cross a
    TILE_COLS boundary. Classes are assigned a reduction mode:
      A: DVE tensor_reduce window L
      B: GpSimd pair-add then DVE tensor_reduce window L/2
      C: (L==2) GpSimd pair-add straight into ssb
    Returns per-tile run lists plus slot metadata."""
    KMAX = int(counts.max())
    n_prog = np.zeros(KMAX + 1, np.int64)
    for c in range(N_CORES):
        bc = np.bincount(counts[c], minlength=KMAX + 1)
        n_prog = np.maximum(n_prog, bc)

    def L_of(k):
        if even_pad:
            return max(2, k + (k & 1))
        return max(1, k)

    classes = [k for k in range(KMAX + 1) if n_prog[k] > 0]
    W = {k: -(-int(n_prog[k]) // 2) for k in classes}
    # mode assignment: hottest (largest L first) classes to GpSimd
    mode = {}
    gp_load = 0.0
    for k in sorted(classes, key=lambda k: -L_of(k)):
        L = L_of(k)
        C_k = W[k] * L
        if L % 2 == 0 and gp_load + C_k * GP_PAIR_NS <= GP_BUDGET_NS:
            mode[k] = "C" if L == 2 else "B"
            gp_load += C_k * GP_PAIR_NS
        else:
            mode[k] = "A"

    # Interleave GpSimd-bound (B/C) and DVE-bound (A) classes through the
    # column schedule so both engines see steady work from the start.
    gp_cls = [k for k in classes if mode[k] != "A"]
    dve_cls = [k for k in classes if mode[k] == "A"]
    gp_tot = sum(W[k] * L_of(k) for k in gp_cls) or 1
    dve_tot = sum(W[k] * L_of(k) for k in dve_cls) or 1
    emit_order = []
    gi = di = 0
    gp_done = dve_done = 0.0
    while gi < len(gp_cls) or di < len(dve_cls):
        if di >= len(dve_cls) or (
            gi < len(gp_cls) and gp_done / gp_tot <= dve_done / dve_tot
        ):
            k = gp_cls[gi]; gi += 1
            gp_done += W[k] * L_of(k)
        else:
            k = dve_cls[di]; di += 1
            dve_done += W[k] * L_of(k)
        emit_order.append(k)

    pair_runs = {}  # tile -> [(off, ncols, dst0, to_ssb)]
    red_runs = {}  # tile -> [(src_h2, off, n, Lw, slot0)]
    slot_col = []
    cls_slot0 = {}
    col = 0
    for k in emit_order:
        L = L_of(k)
        cls_slot0[k] = len(slot_col)
        left = W[k]
        while left:
            space = TILE_COLS - (col % TILE_COLS)
            nfit = min(left, space // L)
            if nfit == 0:
                col += space
                continue
            t = col // TILE_COLS
            off = col - t * TILE_COLS
            slot0 = len(slot_col)
            if mode[k] == "A":
                red_runs.setdefault(t, []).append((False, off, nfit, L, slot0))
            elif mode[k] == "C":
                pair_runs.setdefault(t, []).append((off, nfit * L, slot0, True))
            else:  # B
                pair_runs.setdefault(t, []).append((off, nfit * L, off // 2, False))
                red_runs.setdefault(t, []).append(
                    (True, off // 2, nfit, L // 2, slot0)
                )
            slot_col.extend((col + L * np.arange(nfit)).tolist())
            col += nfit * L
            left -= nfit
    NSLOT = len(slot_col)
    NSLOT_pad = -(-NSLOT // RHO_W) * RHO_W
    NTILES = -(-col // TILE_COLS)
    S = NTILES * TILE_COLS
    Lmap = {k: L_of(k) for k in classes}
    return (pair_runs, red_runs, np.asarray(slot_col, np.int64), cls_slot0,
            Lmap, NSLOT, NSLOT_pad, S, NTILES)


def _build_program(pair_runs, red_runs, NTILES, NSLOT_pad, S):
    from concourse import bacc, mybir
    import concourse.tile as tile

    FP16 = mybir.dt.float16
    F32 = mybir.dt.float32
    Relu = mybir.ActivationFunctionType.Relu
    Copy = mybir.ActivationFunctionType.Copy
    Identity = mybir.ActivationFunctionType.Identity
    AXX = mybir.AxisListType.X
    ADD = mybir.AluOpType.add
    MAX = mybir.AluOpType.max

    nc = bacc.Bacc("TRN2", target_bir_lowering=False, debug=False)
    xd = nc.dram_tensor("xd", [128, S], FP16, kind="ExternalInput").ap()
    cntd = nc.dram_tensor("cntd", [65, NSLOT_pad], FP16, kind="ExternalInput").ap()
    w2sd = nc.dram_tensor("w2sd", [128, 128], FP16, kind="ExternalInput").ap()
    b1d = nc.dram_tensor("b1d", [128, 1], F32, kind="ExternalInput").ap()
    waad = nc.dram_tensor("waad", [128, 64], FP16, kind="ExternalInput").ap()
    b2rd = nc.dram_tensor("b2rd", [65, 64], FP16, kind="ExternalInput").ap()
    rb1d = nc.dram_tensor("rb1d", [64, 1], F32, kind="ExternalInput").ap()
    rw2d = nc.dram_tensor("rw2d", [64, 2], FP16, kind="ExternalInput").ap()
    rb2d = nc.dram_tensor("rb2d", [2, 1], F32, kind="ExternalInput").ap()
    outd = nc.dram_tensor("outd", [4, NSLOT_pad], F32, kind="ExternalOutput").ap()

    NWIN = NSLOT_pad // RHO_W
    with tile.TileContext(nc) as tc:
        with (
            tc.tile_pool(name="const", bufs=1) as cpool,
            tc.tile_pool(name="x", bufs=3) as xpool,
            tc.tile_pool(name="h", bufs=3) as hpool,
            tc.tile_pool(name="h2", bufs=3) as h2pool,
            tc.tile_pool(name="r", bufs=2) as rpool,
            tc.tile_pool(name="ph", bufs=3, space="PSUM") as ph,
            tc.tile_pool(name="pr", bufs=1, space="PSUM") as pr,
            tc.tile_pool(name="po", bufs=1, space="PSUM") as po,
        ):
            # mm1-critical consts first so tile 0's matmul starts ASAP;
            # rho-stage consts are DMA'd after tile 0 is in flight.
            w2s_t = cpool.tile([128, 128], FP16)
            nc.sync.dma_start(w2s_t[:], w2sd[:, :])
            b1_t = cpool.tile([128, 1], F32)
            nc.sync.dma_start(b1_t[:], b1d[:, :])
            waa_t = cpool.tile([128, 64], FP16)
            b2r_t = cpool.tile([65, 64], FP16)
            rb1_t = cpool.tile([64, 1], F32)
            rw2_t = cpool.tile([64, 2], FP16)
            rb2_t = cpool.tile([2, 1], F32)
            cnt_t = cpool.tile([65, NSLOT_pad], FP16)
            ssb = cpool.tile([128, NSLOT_pad], FP16)
            outsb = cpool.tile([66, NSLOT_pad], F32)
            nc.gpsimd.memset(ssb[:], 0.0)

            with nc.allow_low_precision(reason="fp16 segment sums within tol"):
                for t in range(NTILES):
                    xt = xpool.tile([128, TILE_COLS], FP16)
                    nc.sync.dma_start(
                        xt[:], xd[:, t * TILE_COLS : (t + 1) * TILE_COLS]
                    )
                    if t == 1:
                        nc.sync.dma_start(waa_t[:], waad[:, :])
                        nc.sync.dma_start(b2r_t[:], b2rd[:, :])
                        nc.sync.dma_start(rb1_t[:], rb1d[:, :])
                        nc.sync.dma_start(rw2_t[:], rw2d[:, :])
                        nc.sync.dma_start(rb2_t[:], rb2d[:, :])
                        nc.sync.dma_start(cnt_t[:], cntd[:, :])
                    hst = hpool.tile([128, TILE_COLS], FP16)
                    h2t = h2pool.tile([128, TILE_COLS // 2], FP16)
                    for g in range(TILE_COLS // (2 * SUB)):
                        hp = ph.tile([128, 2 * SUB], F32)
                        for j in range(2):
                            nc.tensor.matmul(
                                hp[:, j * SUB : (j + 1) * SUB],
                                lhsT=w2s_t[:],
                                rhs=xt[:, (2 * g + j) * SUB : (2 * g + j + 1) * SUB],
                                start=True,
                                stop=True,
                            )
                        c0g = 2 * g * SUB
                        if g == 0:
                            nc.scalar.activation(
                                hst[:, c0g : c0g + 2 * SUB], hp[:], Relu,
                                bias=b1_t[:],
                            )
                        else:
                            nc.scalar.activation(
                                hst[:, c0g : c0g + SUB], hp[:, 0:SUB], Relu,
                                bias=b1_t[:],
                            )
                            nc.vector.tensor_scalar(
                                out=hst[:, c0g + SUB : c0g + 2 * SUB],
                                in0=hp[:, SUB : 2 * SUB], scalar1=b1_t[:],
                                scalar2=0.0, op0=ADD, op1=MAX,
                            )
                    for (off, ncols, dst0, to_ssb) in pair_runs.get(t, []):
                        i0 = hst[:, off : off + ncols : 2]
                        i1 = hst[:, off + 1 : off + ncols : 2]
                        out_ap = (
                            ssb[:, dst0 : dst0 + ncols // 2]
                            if to_ssb
                            else h2t[:, dst0 : dst0 + ncols // 2]
                        )
                        nc.gpsimd.tensor_tensor(
                            out=out_ap, in0=i0, in1=i1, op=ADD
                        )
                    for (src_h2, off, n, Lw, slot0) in red_runs.get(t, []):
                        src = h2t if src_h2 else hst
                        nc.vector.tensor_reduce(
                            ssb[:, slot0 : slot0 + n],
                            src[:, off : off + n * Lw].rearrange(
                                "p (n l) -> p n l", l=Lw
                            ),
                            axis=AXX,
                            op=ADD,
                        )
                for w in range(NWIN):
                    c0 = w * RHO_W
                    for h in range(2):
                        rp = pr.tile([64, RHO_W], F32)
                        nc.tensor.matmul(
                            rp[:], lhsT=waa_t[64 * h : 64 * h + 64, :],
                            rhs=ssb[64 * h : 64 * h + 64, c0 : c0 + RHO_W],
                            start=True, stop=False,
                        )
                        nc.tensor.matmul(
                            rp[:], lhsT=b2r_t[64 * h : 64 * h + 1, :],
                            rhs=cnt_t[64 * h : 64 * h + 1, c0 : c0 + RHO_W],
                            start=False, stop=True,
                        )
                        rsb = rpool.tile([64, RHO_W], FP16)
                        nc.scalar.activation(rsb[:], rp[:], Relu, bias=rb1_t[:])
                        op_ = po.tile([2, RHO_W], F32)
                        nc.tensor.matmul(
                            op_[:], lhsT=rw2_t[:], rhs=rsb[:],
                            start=True, stop=True,
                        )
                        nc.scalar.activation(
                            outsb[64 * h : 64 * h + 2, c0 : c0 + RHO_W], op_[:],
                            Identity, bias=rb2_t[:],
                        )
                    nc.sync.dma_start(
                        outd[0:2, c0 : c0 + RHO_W], outsb[0:2, c0 : c0 + RHO_W]
                    )
                    nc.sync.dma_start(
                        outd[2:4, c0 : c0 + RHO_W], outsb[64:66, c0 : c0 + RHO_W]
                    )
    nc.compile()
    return nc


def _host_prep(neighbors, phi_w1, phi_b1, phi_w2, phi_b2,
               rho_w1, rho_b1, rho_w2, rho_b2, segment_ids):
    ids = np.asarray(segment_ids)
    X16 = np.asarray(neighbors).astype(np.float16)

    counts_all = np.bincount(ids, minlength=N_AGENTS)
    row_start = np.concatenate([[0], np.cumsum(counts_all)])
    # deal segments round-robin by size so core class-histograms match
    order_all = np.argsort(counts_all, kind="stable")
    core_segs = [order_all[c::N_CORES] for c in range(N_CORES)]
    maxn = max(len(s) for s in core_segs)
    counts = np.zeros((N_CORES, maxn), np.int64)
    for c in range(N_CORES):
        counts[c, : len(core_segs[c])] = counts_all[core_segs[c]]

    even_pad = bool(np.all(np.asarray(phi_b1) == 0))
    sched = _make_schedule(counts, even_pad)
    (pair_runs, red_runs, slot_col, cls_slot0, Lmap,
     NSLOT, NSLOT_pad, S, NTILES) = sched

    w2s = np.zeros((128, 128), np.float16)
    w2s[0:64, 0:64] = phi_w1
    w2s[64:128, 64:128] = phi_w1
    consts = dict(
        w2sd=w2s,
        b1d=np.tile(np.asarray(phi_b1, np.float32)[:, None], (2, 1)),
        waad=np.tile((np.asarray(phi_w2) @ np.asarray(rho_w1)).astype(np.float16),
                     (2, 1)),
        b2rd=np.zeros((65, 64), np.float16),
        rb1d=np.asarray(rho_b1, np.float32)[:, None],
        rw2d=np.asarray(rho_w2).astype(np.float16),
        rb2d=np.asarray(rho_b2, np.float32)[:, None],
    )
    b2row = (np.asarray(phi_b2) @ np.asarray(rho_w1)).astype(np.float16)
    consts["b2rd"][0] = b2row
    consts["b2rd"][64] = b2row

    in_maps = []
    slotmaps = []
    for c in range(N_CORES):
        segs_c = core_segs[c]  # ascending by count (argsort order)
        k_c = counts_all[segs_c]
        uniq, first = np.unique(k_c, return_index=True)
        xdv = np.zeros((128, S), np.float16)
        cnt = np.zeros((65, NSLOT_pad), np.float16)
        smap = np.full((2, NSLOT_pad), -1, np.int64)
        for h in (0, 1):
            seg_list, slot_list = [], []
            for i, kv in enumerate(uniq):
                s0 = first[i]
                s1 = first[i + 1] if i + 1 < len(uniq) else len(segs_c)
                segs = segs_c[s0:s1]
                nE = (len(segs) + 1) // 2
                mine = segs[:nE] if h == 0 else segs[nE:]
                if len(mine) == 0:
                    continue
                base = cls_slot0[int(kv)]
                seg_list.append(mine)
                slot_list.append(base + np.arange(len(mine)))
            segs_f = np.concatenate(seg_list)
            slots_f = np.concatenate(slot_list)
            kk = counts_all[segs_f]
            nz = kk > 0
            segs_nz, slots_nz, kknz = segs_f[nz], slots_f[nz], kk[nz]
            src0 = row_start[segs_nz]
            col0 = slot_col[slots_nz]
            tot = int(kknz.sum())
            ofs = np.arange(tot) - np.repeat(np.cumsum(kknz) - kknz, kknz)
            src = np.repeat(src0, kknz) + ofs
            dst = np.repeat(col0, kknz) + ofs
            xdv[64 * h : 64 * h + 64, dst] = X16[src].T
            cnt[64 * h, slots_f] = kk.astype(np.float16)
            smap[h, slots_f] = segs_f
        in_maps.append(dict(xd=xdv, cntd=cnt, **consts))
        slotmaps.append(smap)
    return sched, in_maps, slotmaps


def kernel(**inputs):
    global LAST_RESULTS
    np_inputs = {kk: np.asarray(v) for kk, v in inputs.items()}
    sched, in_maps, slotmaps = _host_prep(**np_inputs)
    (pair_runs, red_runs, slot_col, cls_slot0, Lmap,
     NSLOT, NSLOT_pad, S, NTILES) = sched
    nc = _build_program(pair_runs, red_runs, NTILES, NSLOT_pad, S)

    from concourse.bass_utils import run_bass_kernel_spmd

    res = run_bass_kernel_spmd(nc, in_maps, list(range(N_CORES)))
    LAST_RESULTS = res

    y = np.zeros((N_AGENTS, 2), np.float32)
    for c in range(N_CORES):
        o = res.results[c]["outd"]
        sm = slotmaps[c]
        for h in (0, 1):
            m = sm[h] >= 0
            y[sm[h][m]] = o[2 * h : 2 * h + 2, m].T
    return y
